# revision 3
# baseline (speedup 1.0000x reference)
"""NoPE attention block (QKV proj -> causal attention -> dense) on 8 TRN2 cores.

Sharding: tensor-parallel over heads. Each of the 8 cores computes 2 of the 16
heads end-to-end (its slice of the QKV projection, full causal attention for
those heads, and the corresponding 256 rows of the dense projection), producing
a partial [4096, 2048] output. The host sums the 8 partials and adds b_dense.

Self-contained: only needs numpy/ml_dtypes/jax/concourse (all on PYTHONPATH).
"""

import numpy as np
import ml_dtypes

B, S, H, NH = 2, 2048, 2048, 16
HD = 128
NCORES = 8
NL = NH // NCORES            # heads per core = 2
T = B * S                    # 4096
SCALE = float(HD) ** -0.5

_BF16 = ml_dtypes.bfloat16

_CACHE = {}


def _build_nc(reps=1, h=H, s=S, b=B):
    """Build + compile the per-core Bass program.

    reps>1 wraps the whole body in a hardware For_i loop (for slope timing).
    h/s/b can be shrunk for fast simulator tests.
    """
    import concourse.bass as bass  # noqa: F401
    import concourse.mybir as mybir
    import concourse.tile as tile
    from concourse import bacc
    from concourse.bass import ts, ds
    from concourse.masks import make_identity, make_upper_triangular

    f32 = mybir.dt.float32
    bf16 = mybir.dt.bfloat16
    FT = mybir.ActivationFunctionType

    t_tot = b * s
    KS = h // 128            # contraction subtiles
    TC = 512                 # t-chunk for QKV
    NTC = t_tot // TC
    QT_N = s // 128          # q tiles per batch
    TT_N = t_tot // 128      # t tiles overall
    NCOL = max(1, h // 512)  # dense output column chunks

    nc = bacc.Bacc("TRN2", target_bir_lowering=False, debug=False)

    hid_t = nc.dram_tensor("hid_t", [h, t_tot], bf16, kind="ExternalInput").ap()
    wqk = nc.dram_tensor("wqk", [h, 2 * NL * HD], bf16, kind="ExternalInput").ap()
    wv = nc.dram_tensor("wv", [h, NL * HD], bf16, kind="ExternalInput").ap()
    wd = nc.dram_tensor("wd", [NL * HD, h], bf16, kind="ExternalInput").ap()
    bqk = nc.dram_tensor("bqk", [HD, 2 * NL], f32, kind="ExternalInput").ap()
    bvd = nc.dram_tensor("bvd", [HD, NL * HD], f32, kind="ExternalInput").ap()
    out = nc.dram_tensor("out_partial", [t_tot, h], bf16, kind="ExternalOutput").ap()

    hid_r = hid_t.rearrange("(ho p) t -> p ho t", p=128)
    wqk_r = wqk.rearrange("(ho p) j -> p ho j", p=128)
    wv_r = wv.rearrange("(ho p) j -> p ho j", p=128)
    wd_r = wd.rearrange("(ho p) n -> p ho n", p=128)
    out_r = out.rearrange("(to p) n -> p to n", p=128)

    with tile.TileContext(nc) as tc:
        with (
            tc.tile_pool(name="const", bufs=1) as const,
            tc.tile_pool(name="hid", bufs=2) as hidp,
            tc.tile_pool(name="work", bufs=3) as work,
            tc.tile_pool(name="psum", bufs=2, space="PSUM") as psum,
        ):
            # ---- constants (loaded once, outside the reps loop) ----
            wqk_sb = const.tile([128, KS, 2 * NL * HD], bf16, tag="wqk")
            nc.sync.dma_start(wqk_sb[:], wqk_r)
            wv_sb = const.tile([128, KS, NL * HD], bf16, tag="wv")
            nc.sync.dma_start(wv_sb[:], wv_r)
            wd_sb = const.tile([128, NL, h], bf16, tag="wd")
            nc.sync.dma_start(wd_sb[:], wd_r)
            bqk_sb = const.tile([128, 2 * NL], f32, tag="bqk")
            nc.sync.dma_start(bqk_sb[:], bqk)
            bvd_sb = const.tile([128, NL * HD], f32, tag="bvd")
            nc.sync.dma_start(bvd_sb[:], bvd)
            mask_sb = const.tile([128, 128], bf16, tag="mask")
            make_upper_triangular(nc, mask_sb[:], val=1.0, diag=True)
            ident_sb = const.tile([128, 128], bf16, tag="ident")
            make_identity(nc, ident_sb[:])

            def body(_i):
                # persistent intermediates (per rep)
                qkT_sb = const.tile([128, 2 * NL, t_tot], bf16, tag="qkT")
                v_sb = const.tile([128, TT_N, NL, HD + 1], bf16, tag="v")
                attnT_sb = const.tile([128, NL, t_tot], bf16, tag="attnT")

                # ones column for fused softmax denominator
                nc.vector.memset(v_sb[:, :, :, HD : HD + 1], 1.0)

                # ---- Phase A: QKV projection ----
                for tci in range(NTC):
                    ht = hidp.tile([128, KS, TC], bf16, tag="ht")
                    nc.sync.dma_start(ht[:], hid_r[:, :, ts(tci, TC)])
                    # q^T / k^T:  psum[j, t] = sum_h W[h, j] * hidden^T[h, t]
                    for j in range(2 * NL):
                        ps = psum.tile([128, TC], f32, tag="a")
                        for hs in range(KS):
                            nc.tensor.matmul(
                                ps[:],
                                lhsT=wqk_sb[:, hs, ts(j, HD)],
                                rhs=ht[:, hs, :],
                                start=(hs == 0),
                                stop=(hs == KS - 1),
                            )
                        nc.vector.tensor_scalar_add(
                            qkT_sb[:, j, ts(tci, TC)], ps[:], bqk_sb[:, j : j + 1]
                        )
                    # v:  psum[t, j] = sum_h hidden^T[h, t] * Wv[h, j]
                    for tt in range(TC // 128):
                        ps = psum.tile([128, NL * HD], f32, tag="a")
                        for hs in range(KS):
                            nc.tensor.matmul(
                                ps[:],
                                lhsT=ht[:, hs, ts(tt, 128)],
                                rhs=wv_sb[:, hs, :],
                                start=(hs == 0),
                                stop=(hs == KS - 1),
                            )
                        ti = tci * (TC // 128) + tt
                        for hh in range(NL):
                            nc.vector.tensor_tensor(
                                v_sb[:, ti, hh, 0:HD],
                                ps[:, ts(hh, HD)],
                                bvd_sb[:, ts(hh, HD)],
                                mybir.AluOpType.add,
                            )

                # ---- Phase B: causal attention per (batch, head, q-tile) ----
                for bb in range(b):
                    for hh in range(NL):
                        for qt in range(QT_N):
                            pvs = psum.tile([128, HD + 1], f32, tag="pv")
                            for kt in range(qt + 1):
                                sp = psum.tile([128, 128], f32, tag="s")
                                nc.tensor.matmul(
                                    sp[:],
                                    lhsT=qkT_sb[:, NL + hh, ds(bb * s + kt * 128, 128)],
                                    rhs=qkT_sb[:, hh, ds(bb * s + qt * 128, 128)],
                                    start=True,
                                    stop=True,
                                )
                                e = work.tile([128, 128], bf16, tag="e")
                                nc.scalar.activation(e[:], sp[:], FT.Exp, scale=SCALE)
                                if kt == qt:
                                    nc.vector.tensor_tensor(
                                        e[:], e[:], mask_sb[:], mybir.AluOpType.mult
                                    )
                                kg = (bb * s) // 128 + kt
                                nc.tensor.matmul(
                                    pvs[:],
                                    lhsT=e[:],
                                    rhs=v_sb[:, kg, hh, :],
                                    start=(kt == 0),
                                    stop=(kt == qt),
                                )
                            rec = work.tile([128, 1], f32, tag="rec")
                            nc.vector.reciprocal(rec[:], pvs[:, HD : HD + 1])
                            a_sb = work.tile([128, 128], bf16, tag="attn")
                            nc.vector.tensor_scalar_mul(a_sb[:], pvs[:, 0:HD], rec[:])
                            tp = psum.tile([128, 128], bf16, tag="s")
                            nc.tensor.transpose(tp[:], a_sb[:], ident_sb[:])
                            nc.scalar.activation(
                                attnT_sb[:, hh, ds(bb * s + qt * 128, 128)],
                                tp[:],
                                FT.Copy,
                            )

                # ---- Phase C: dense projection (partial over this core's heads) ----
                for tt in range(TT_N):
                    for ncc in range(NCOL):
                        dps = psum.tile([128, min(512, h)], f32, tag="d")
                        for hh in range(NL):
                            nc.tensor.matmul(
                                dps[:],
                                lhsT=attnT_sb[:, hh, ts(tt, 128)],
                                rhs=wd_sb[:, hh, ts(ncc, min(512, h))],
                                start=(hh == 0),
                                stop=(hh == NL - 1),
                            )
                        dst = work.tile([128, min(512, h)], f32, tag="dst")
                        nc.any.tensor_copy(dst[:], dps[:])
                        nc.sync.dma_start(out_r[:, tt, ts(ncc, min(512, h))], dst[:])

            if reps == 1:
                body(0)
            else:
                with tc.For_i(0, reps, 1) as i:
                    body(i)

    nc.compile()
    return nc


def _pack_inputs(hidden_states, W_qkv, b_qkv, W_dense):
    """Per-core input maps (host-side sharding)."""
    hid = np.asarray(hidden_states, dtype=np.float32).reshape(T, H)
    hid_t = np.ascontiguousarray(hid.T).astype(_BF16)
    W_qkv = np.asarray(W_qkv, dtype=np.float32)
    b_qkv = np.asarray(b_qkv, dtype=np.float32)
    W_dense = np.asarray(W_dense, dtype=np.float32)
    Wq, Wk, Wv = W_qkv[:, 0:H], W_qkv[:, H : 2 * H], W_qkv[:, 2 * H : 3 * H]
    bq, bk, bv = b_qkv[0:H], b_qkv[H : 2 * H], b_qkv[2 * H : 3 * H]

    in_maps = []
    for c in range(NCORES):
        hs = [NL * c + i for i in range(NL)]
        cols = [Wq[:, h * HD : (h + 1) * HD] for h in hs] + [
            Wk[:, h * HD : (h + 1) * HD] for h in hs
        ]
        wqk_c = np.concatenate(cols, axis=1).astype(_BF16)
        wv_c = np.concatenate(
            [Wv[:, h * HD : (h + 1) * HD] for h in hs], axis=1
        ).astype(_BF16)
        wd_c = np.ascontiguousarray(
            W_dense[c * NL * HD : (c + 1) * NL * HD, :]
        ).astype(_BF16)
        bqk_c = np.stack(
            [bq[h * HD : (h + 1) * HD] for h in hs]
            + [bk[h * HD : (h + 1) * HD] for h in hs],
            axis=1,
        ).astype(np.float32)
        bv_row = np.concatenate([bv[h * HD : (h + 1) * HD] for h in hs])
        bvd_c = np.ascontiguousarray(
            np.broadcast_to(bv_row[None, :], (HD, NL * HD))
        ).astype(np.float32)
        in_maps.append(
            {
                "hid_t": hid_t,
                "wqk": wqk_c,
                "wv": wv_c,
                "wd": wd_c,
                "bqk": bqk_c,
                "bvd": bvd_c,
            }
        )
    return in_maps


def make_runner(nc, n_cores=NCORES):
    """Reusable jitted SPMD runner (no donation; device-resident inputs)."""
    import jax
    import concourse.mybir as mybir
    from jax.sharding import Mesh, PartitionSpec
    from jax.experimental.shard_map import shard_map
    from concourse.bass2jax import (
        _bass_exec_p,
        partition_id_tensor,
        install_neuronx_cc_hook,
    )

    install_neuronx_cc_hook()
    partition_name = nc.partition_id_tensor.name if nc.partition_id_tensor else None
    in_names, out_names, out_avals = [], [], []
    for alloc in nc.m.functions[0].allocations:
        if not isinstance(alloc, mybir.MemoryLocationSet):
            continue
        name = alloc.memorylocations[0].name
        if alloc.kind == "ExternalInput":
            if name != partition_name:
                in_names.append(name)
        elif alloc.kind == "ExternalOutput":
            out_names.append(name)
            out_avals.append(
                jax.core.ShapedArray(
                    tuple(alloc.tensor_shape), mybir.dt.np(alloc.dtype)
                )
            )
    n_params = len(in_names)
    all_in_names = list(in_names) + list(out_names)
    if partition_name is not None:
        all_in_names.append(partition_name)
    zero_outs = [np.zeros(a.shape, a.dtype) for a in out_avals]

    def _body(*args):
        operands = list(args)
        if partition_name is not None:
            operands.append(partition_id_tensor())
        outs = _bass_exec_p.bind(
            *operands,
            out_avals=tuple(out_avals),
            in_names=tuple(all_in_names),
            out_names=tuple(out_names),
            lowering_input_output_aliases=(),
            sim_require_finite=True,
            sim_require_nnan=True,
            nc=nc,
        )
        return tuple(outs)

    devices = jax.devices()[:n_cores]
    mesh = Mesh(np.asarray(devices), ("core",))
    in_specs = (PartitionSpec("core"),) * (n_params + len(out_names))
    out_specs = (PartitionSpec("core"),) * len(out_names)
    sharded = jax.jit(
        shard_map(
            _body, mesh=mesh, in_specs=in_specs, out_specs=out_specs, check_rep=False
        ),
        keep_unused=True,
    )

    def prepare(in_maps):
        per_core = [[np.asarray(m[name]) for name in in_names] for m in in_maps]
        concat_in = [
            np.concatenate([per_core[c][i] for c in range(n_cores)], axis=0)
            for i in range(n_params)
        ]
        concat_zero = [
            np.zeros((n_cores * z.shape[0], *z.shape[1:]), z.dtype) for z in zero_outs
        ]
        return [jax.device_put(a) for a in concat_in + concat_zero]

    def run(dev_args):
        outs = sharded(*dev_args)
        jax.block_until_ready(outs)
        return outs

    def fetch(outs):
        return [
            {
                name: np.asarray(outs[i]).reshape(n_cores, *out_avals[i].shape)[c]
                for i, name in enumerate(out_names)
            }
            for c in range(n_cores)
        ]

    return prepare, run, fetch


def kernel(hidden_states, W_qkv, b_qkv, W_dense, b_dense):
    from concourse import bass_utils

    if "nc1" not in _CACHE:
        _CACHE["nc1"] = _build_nc(reps=1)
    nc = _CACHE["nc1"]

    in_maps = _pack_inputs(hidden_states, W_qkv, b_qkv, W_dense)
    res = bass_utils.run_bass_kernel_spmd(nc, in_maps, core_ids=list(range(NCORES)))

    acc = np.zeros((T, H), dtype=np.float64)
    for c in range(NCORES):
        acc += res.results[c]["out_partial"].astype(np.float64)
    acc += np.asarray(b_dense, dtype=np.float64)[None, :]
    return acc.astype(np.float32).reshape(B, S, H)


# revision 30
# speedup vs baseline: 1.1470x; 1.1470x over previous
"""NoPE attention block (QKV proj -> causal attention -> dense) on 8 TRN2 cores.

Sharding: tensor-parallel over heads. Each of the 8 cores computes 2 of the 16
heads end-to-end (its slice of the QKV projection, full causal attention for
those heads, and the corresponding 256 rows of the dense projection), producing
a partial [4096, 2048] output. The host sums the 8 partials and adds b_dense.

Self-contained: only needs numpy/ml_dtypes/jax/concourse (all on PYTHONPATH).
"""

import numpy as np
import ml_dtypes

B, S, H, NH = 2, 2048, 2048, 16
HD = 128
NCORES = 8
NL = NH // NCORES            # heads per core = 2
T = B * S                    # 4096
SCALE = float(HD) ** -0.5

_BF16 = ml_dtypes.bfloat16

_CACHE = {}


def _build_nc(reps=1, h=H, s=S, b=B):
    """Build + compile the per-core Bass program.

    reps>1 wraps the whole body in a hardware For_i loop (for slope timing).
    h/s/b can be shrunk for fast simulator tests.
    """
    import concourse.bass as bass  # noqa: F401
    import concourse.mybir as mybir
    import concourse.tile as tile
    from concourse import bacc
    from concourse.bass import ts, ds
    from concourse.masks import make_identity, make_upper_triangular

    f32 = mybir.dt.float32
    bf16 = mybir.dt.bfloat16
    FT = mybir.ActivationFunctionType

    t_tot = b * s
    KS = h // 128            # contraction subtiles
    TC = 512                 # t-chunk for QKV
    NTC = t_tot // TC
    QT_N = s // 128          # q tiles per batch
    TT_N = t_tot // 128      # t tiles overall
    NCOL = max(1, h // 512)  # dense output column chunks

    nc = bacc.Bacc("TRN2", target_bir_lowering=False, debug=False)

    hid_t = nc.dram_tensor("hid_t", [h, t_tot], bf16, kind="ExternalInput").ap()
    wqk = nc.dram_tensor("wqk", [h, 2 * NL * HD], bf16, kind="ExternalInput").ap()
    wv = nc.dram_tensor("wv", [h, NL * HD], bf16, kind="ExternalInput").ap()
    wd = nc.dram_tensor("wd", [NL * HD, h], bf16, kind="ExternalInput").ap()
    bqk = nc.dram_tensor("bqk", [HD, 2 * NL], f32, kind="ExternalInput").ap()
    bvd = nc.dram_tensor("bvd", [HD, NL * HD], f32, kind="ExternalInput").ap()
    out = nc.dram_tensor("out_partial", [t_tot, h], bf16, kind="ExternalOutput").ap()

    hid_r = hid_t.rearrange("(ho p) t -> p ho t", p=128)
    wqk_r = wqk.rearrange("(ho p) j -> p ho j", p=128)
    wv_r = wv.rearrange("(ho p) j -> p ho j", p=128)
    wd_r = wd.rearrange("(ho p) n -> p ho n", p=128)
    out_r = out.rearrange("(to p) n -> p to n", p=128)

    with tile.TileContext(nc) as tc:
        with (
            tc.tile_pool(name="const", bufs=1) as const,
            tc.tile_pool(name="hid", bufs=3) as hidp,
            tc.tile_pool(name="work", bufs=3) as work,
            tc.tile_pool(name="psum", bufs=2, space="PSUM") as psum,
        ):
            # ---- constants (loaded once, outside the reps loop) ----
            wqk_sb = const.tile([128, KS, 2 * NL * HD], bf16, tag="wqk")
            wv_sb = const.tile([128, KS, NL * HD], bf16, tag="wv")
            pw = KS // 4
            # only the first wqk piece up-front; the rest go after the first
            # hidden piece so the first matmul can start ASAP
            nc.sync.dma_start(wqk_sb[:, ds(0, pw), :], wqk_r[:, ds(0, pw), :])
            wd_sb = const.tile([128, NL, h], bf16, tag="wd")
            bqk_sb = const.tile([128, 2 * NL], f32, tag="bqk")
            nc.sync.dma_start(bqk_sb[:], bqk)
            bvd_sb = const.tile([128, NL * HD], f32, tag="bvd")
            nc.sync.dma_start(bvd_sb[:], bvd)
            # triangle mask: tri[k, q] = 1 if k <= q else 0 (for diagonal tiles)
            tri_sb = const.tile([128, 128], bf16, tag="tri")
            make_upper_triangular(nc, tri_sb[:], val=1.0, diag=True)
            ident_sb = const.tile([128, 128], bf16, tag="ident")
            make_identity(nc, ident_sb[:])

            QW = 512
            QC_N = s // QW
            NW = min(512, h)

            def body(_i):
                # persistent intermediates (per rep)
                qkT_sb = const.tile([128, 2 * NL, t_tot], bf16, tag="qkT")
                v_sb = const.tile([128, TT_N, NL, HD + 1], bf16, tag="v")
                attnT_sb = const.tile([128, NL, t_tot], bf16, tag="attnT")

                # ones column for fused softmax denominator
                nc.vector.memset(v_sb[:, :, :, HD : HD + 1], 1.0)

                def qkv_chunk(tci):
                    ht = hidp.tile([128, KS, TC], bf16, tag="ht")
                    # split the chunk DMA so the first matmuls can start sooner
                    # (subtile deps let hs-slice consumers fire per piece)
                    nc.sync.dma_start(
                        ht[:, ds(0, KS // 4), :],
                        hid_r[:, ds(0, KS // 4), ts(tci, TC)],
                    )
                    if tci == 0:
                        # remaining weight pieces, after the first hidden piece
                        for pc in range(1, 4):
                            nc.sync.dma_start(
                                wqk_sb[:, ds(pc * pw, pw), :],
                                wqk_r[:, ds(pc * pw, pw), :],
                            )
                    for pc in range(1, 4):
                        nc.sync.dma_start(
                            ht[:, ds(pc * (KS // 4), KS // 4), :],
                            hid_r[:, ds(pc * (KS // 4), KS // 4), ts(tci, TC)],
                        )
                    if tci == 0:
                        for pc in range(4):
                            nc.sync.dma_start(
                                wv_sb[:, ds(pc * pw, pw), :],
                                wv_r[:, ds(pc * pw, pw), :],
                            )
                    # q^T / k^T:  psum[j, t] = sum_h W[h, j] * hidden^T[h, t]
                    for j in range(2 * NL):
                        ps = psum.tile([128, TC], f32, tag="a", bufs=2)
                        for hs in range(KS):
                            nc.tensor.matmul(
                                ps[:],
                                lhsT=wqk_sb[:, hs, ts(j, HD)],
                                rhs=ht[:, hs, :],
                                start=(hs == 0),
                                stop=(hs == KS - 1),
                            )
                        nc.vector.tensor_scalar_add(
                            qkT_sb[:, j, ts(tci, TC)], ps[:], bqk_sb[:, j : j + 1]
                        )
                    # v:  psum[t, j] = sum_h hidden^T[h, t] * Wv[h, j]
                    for tt in range(TC // 128):
                        ps = psum.tile([128, NL * HD], f32, tag="a", bufs=2)
                        for hs in range(KS):
                            nc.tensor.matmul(
                                ps[:],
                                lhsT=ht[:, hs, ts(tt, 128)],
                                rhs=wv_sb[:, hs, :],
                                start=(hs == 0),
                                stop=(hs == KS - 1),
                            )
                        ti = tci * (TC // 128) + tt
                        for hh in range(NL):
                            nc.vector.tensor_tensor(
                                v_sb[:, ti, hh, 0:HD],
                                ps[:, ts(hh, HD)],
                                bvd_sb[:, ts(hh, HD)],
                                mybir.AluOpType.add,
                            )

                def attn_qc(bb, qc):
                    if True:
                        for hh in range(NL):
                            pvs = [
                                psum.tile(
                                    [128, HD + 1], f32, tag="pv", bufs=4,
                                    name=f"pv{j}",
                                )
                                for j in range(4)
                            ]
                            n_kt = 4 * qc + 4
                            for kt in range(n_kt):
                                d_ = kt - 4 * qc
                                q0 = max(d_, 0) * 128  # skip fully-masked q cols
                                sp = psum.tile([128, QW], f32, tag="s", bufs=2)
                                nc.tensor.matmul(
                                    sp[:, q0:QW],
                                    lhsT=qkT_sb[:, NL + hh, ds(bb * s + kt * 128, 128)],
                                    rhs=qkT_sb[:, hh, ds(bb * s + qc * QW + q0, QW - q0)],
                                    start=True,
                                    stop=True,
                                )
                                e = work.tile([128, QW], bf16, tag="e", bufs=6)
                                nc.scalar.activation(
                                    e[:, q0:QW], sp[:, q0:QW], FT.Exp, scale=SCALE
                                )
                                if d_ >= 0:
                                    nc.vector.tensor_tensor(
                                        e[:, ds(q0, 128)],
                                        e[:, ds(q0, 128)],
                                        tri_sb[:],
                                        mybir.AluOpType.mult,
                                    )
                                kg = (bb * s) // 128 + kt
                                for j in range(max(d_, 0), 4):
                                    qt = 4 * qc + j
                                    nc.tensor.matmul(
                                        pvs[j][:],
                                        lhsT=e[:, ts(j, 128)],
                                        rhs=v_sb[:, kg, hh, :],
                                        start=(kt == 0),
                                        stop=(kt == qt),
                                    )
                            for j in range(4):
                                qt = 4 * qc + j
                                rec = work.tile([128, 1], f32, tag="rec", bufs=6)
                                nc.vector.reciprocal(rec[:], pvs[j][:, HD : HD + 1])
                                a_sb = work.tile([128, 128], bf16, tag="attn", bufs=6)
                                nc.vector.tensor_scalar_mul(
                                    a_sb[:], pvs[j][:, 0:HD], rec[:]
                                )
                                tp = psum.tile([128, 128], bf16, tag="s", bufs=2)
                                nc.tensor.transpose(tp[:], a_sb[:], ident_sb[:])
                                nc.vector.tensor_copy(
                                    attnT_sb[:, hh, ds(bb * s + qt * 128, 128)],
                                    tp[:],
                                )

                def dense_tiles(tts):
                    for tt in tts:
                        for ncc in range(NCOL):
                            dps = psum.tile([128, NW], f32, tag="a", bufs=2)
                            for hh in range(NL):
                                nc.tensor.matmul(
                                    dps[:],
                                    lhsT=attnT_sb[:, hh, ts(tt, 128)],
                                    rhs=wd_sb[:, hh, ts(ncc, NW)],
                                    start=(hh == 0),
                                    stop=(hh == NL - 1),
                                )
                            dst = work.tile([128, NW], bf16, tag="dst", bufs=4)
                            # alternate engines so the psum->sbuf copy rate
                            # doesn't gate the 2-slot psum pipeline
                            if (tt * NCOL + ncc) % 2 == 0:
                                nc.vector.tensor_copy(dst[:], dps[:])
                            else:
                                nc.scalar.copy(dst[:], dps[:])
                            nc.sync.dma_start(out_r[:, tt, ts(ncc, NW)], dst[:])

                # Interleaved emission so ACT-bound attention overlaps PE-bound
                # QKV/dense work (the scheduler's lookahead is bounded, so the
                # interleave must happen at emission order).
                npc = NTC // b  # qkv chunks per batch
                tpb = TT_N // b  # dense t-tiles per batch
                for tci in range(npc):
                    qkv_chunk(tci)
                    if tci == 0:
                        # deferred: keeps the startup DMA queue clear for
                        # the first hidden/weight pieces
                        nc.sync.dma_start(wd_sb[:], wd_r)
                for qc in range(QC_N):
                    attn_qc(0, qc)
                    if b > 1 and qc < npc:
                        qkv_chunk(npc + qc)
                if b > 1:
                    for tci in range(npc + QC_N, NTC):
                        qkv_chunk(tci)
                    qtp = tpb // QC_N
                    for qc in range(QC_N):
                        attn_qc(1, qc)
                        dense_tiles(range(qc * qtp, (qc + 1) * qtp))
                        if qc > 0:
                            dense_tiles(range(tpb + (qc - 1) * qtp, tpb + qc * qtp))
                    dense_tiles(range(tpb + (QC_N - 1) * qtp, TT_N))
                else:
                    dense_tiles(range(TT_N))

            if reps == 1:
                body(0)
            else:
                with tc.For_i(0, reps, 1) as i:
                    body(i)

    nc.compile()
    return nc


def _pack_inputs(hidden_states, W_qkv, b_qkv, W_dense):
    """Per-core input maps (host-side sharding)."""
    hid = np.asarray(hidden_states, dtype=np.float32).reshape(T, H)
    hid_t = np.ascontiguousarray(hid.T).astype(_BF16)
    W_qkv = np.asarray(W_qkv, dtype=np.float32)
    b_qkv = np.asarray(b_qkv, dtype=np.float32)
    W_dense = np.asarray(W_dense, dtype=np.float32)
    Wq, Wk, Wv = W_qkv[:, 0:H], W_qkv[:, H : 2 * H], W_qkv[:, 2 * H : 3 * H]
    bq, bk, bv = b_qkv[0:H], b_qkv[H : 2 * H], b_qkv[2 * H : 3 * H]

    in_maps = []
    for c in range(NCORES):
        hs = [NL * c + i for i in range(NL)]
        cols = [Wq[:, h * HD : (h + 1) * HD] for h in hs] + [
            Wk[:, h * HD : (h + 1) * HD] for h in hs
        ]
        wqk_c = np.concatenate(cols, axis=1).astype(_BF16)
        wv_c = np.concatenate(
            [Wv[:, h * HD : (h + 1) * HD] for h in hs], axis=1
        ).astype(_BF16)
        wd_c = np.ascontiguousarray(
            W_dense[c * NL * HD : (c + 1) * NL * HD, :]
        ).astype(_BF16)
        bqk_c = np.stack(
            [bq[h * HD : (h + 1) * HD] for h in hs]
            + [bk[h * HD : (h + 1) * HD] for h in hs],
            axis=1,
        ).astype(np.float32)
        bv_row = np.concatenate([bv[h * HD : (h + 1) * HD] for h in hs])
        bvd_c = np.ascontiguousarray(
            np.broadcast_to(bv_row[None, :], (HD, NL * HD))
        ).astype(np.float32)
        in_maps.append(
            {
                "hid_t": hid_t,
                "wqk": wqk_c,
                "wv": wv_c,
                "wd": wd_c,
                "bqk": bqk_c,
                "bvd": bvd_c,
            }
        )
    return in_maps


def make_runner(nc, n_cores=NCORES):
    """Reusable jitted SPMD runner (no donation; device-resident inputs)."""
    import jax
    import concourse.mybir as mybir
    from jax.sharding import Mesh, PartitionSpec
    from jax.experimental.shard_map import shard_map
    from concourse.bass2jax import (
        _bass_exec_p,
        partition_id_tensor,
        install_neuronx_cc_hook,
    )

    install_neuronx_cc_hook()
    partition_name = nc.partition_id_tensor.name if nc.partition_id_tensor else None
    in_names, out_names, out_avals = [], [], []
    for alloc in nc.m.functions[0].allocations:
        if not isinstance(alloc, mybir.MemoryLocationSet):
            continue
        name = alloc.memorylocations[0].name
        if alloc.kind == "ExternalInput":
            if name != partition_name:
                in_names.append(name)
        elif alloc.kind == "ExternalOutput":
            out_names.append(name)
            out_avals.append(
                jax.core.ShapedArray(
                    tuple(alloc.tensor_shape), mybir.dt.np(alloc.dtype)
                )
            )
    n_params = len(in_names)
    all_in_names = list(in_names) + list(out_names)
    if partition_name is not None:
        all_in_names.append(partition_name)
    zero_outs = [np.zeros(a.shape, a.dtype) for a in out_avals]

    def _body(*args):
        operands = list(args)
        if partition_name is not None:
            operands.append(partition_id_tensor())
        outs = _bass_exec_p.bind(
            *operands,
            out_avals=tuple(out_avals),
            in_names=tuple(all_in_names),
            out_names=tuple(out_names),
            lowering_input_output_aliases=(),
            sim_require_finite=True,
            sim_require_nnan=True,
            nc=nc,
        )
        return tuple(outs)

    devices = jax.devices()[:n_cores]
    mesh = Mesh(np.asarray(devices), ("core",))
    in_specs = (PartitionSpec("core"),) * (n_params + len(out_names))
    out_specs = (PartitionSpec("core"),) * len(out_names)
    sharded = jax.jit(
        shard_map(
            _body, mesh=mesh, in_specs=in_specs, out_specs=out_specs, check_rep=False
        ),
        keep_unused=True,
    )

    def prepare(in_maps):
        per_core = [[np.asarray(m[name]) for name in in_names] for m in in_maps]
        concat_in = [
            np.concatenate([per_core[c][i] for c in range(n_cores)], axis=0)
            for i in range(n_params)
        ]
        concat_zero = [
            np.zeros((n_cores * z.shape[0], *z.shape[1:]), z.dtype) for z in zero_outs
        ]
        return [jax.device_put(a) for a in concat_in + concat_zero]

    def run(dev_args):
        outs = sharded(*dev_args)
        jax.block_until_ready(outs)
        return outs

    def fetch(outs):
        return [
            {
                name: np.asarray(outs[i]).reshape(n_cores, *out_avals[i].shape)[c]
                for i, name in enumerate(out_names)
            }
            for c in range(n_cores)
        ]

    return prepare, run, fetch


def kernel(hidden_states, W_qkv, b_qkv, W_dense, b_dense):
    from concourse import bass_utils

    if "nc1" not in _CACHE:
        _CACHE["nc1"] = _build_nc(reps=1)
    nc = _CACHE["nc1"]

    in_maps = _pack_inputs(hidden_states, W_qkv, b_qkv, W_dense)
    res = bass_utils.run_bass_kernel_spmd(nc, in_maps, core_ids=list(range(NCORES)))

    acc = np.zeros((T, H), dtype=np.float64)
    for c in range(NCORES):
        acc += res.results[c]["out_partial"].astype(np.float64)
    acc += np.asarray(b_dense, dtype=np.float64)[None, :]
    return acc.astype(np.float32).reshape(B, S, H)


# revision 38
# speedup vs baseline: 2.2182x; 1.9340x over previous
"""NoPE attention block (QKV proj -> causal attention -> dense) on 8 TRN2 cores.

Sharding: tensor-parallel over heads. Each of the 8 cores computes 2 of the 16
heads end-to-end (its slice of the QKV projection, full causal attention for
those heads, and the corresponding 256 rows of the dense projection), producing
a partial [4096, 2048] output. The host sums the 8 partials and adds b_dense.

Self-contained: only needs numpy/ml_dtypes/jax/concourse (all on PYTHONPATH).
"""

import numpy as np
import ml_dtypes

B, S, H, NH = 2, 2048, 2048, 16
HD = 128
NCORES = 8
NL = NH // NCORES            # heads per core = 2
T = B * S                    # 4096
SCALE = float(HD) ** -0.5

_BF16 = ml_dtypes.bfloat16

_CACHE = {}


def _build_nc(reps=1, h=H, s=S, b=B, phases=("qkv", "attn", "dense")):
    """Build + compile the per-core Bass program.

    reps>1 wraps the whole body in a hardware For_i loop (for slope timing).
    h/s/b can be shrunk for fast simulator tests.
    """
    import concourse.bass as bass  # noqa: F401
    import concourse.mybir as mybir
    import concourse.tile as tile
    from concourse import bacc
    from concourse.bass import ts, ds
    from concourse.masks import make_identity, make_upper_triangular

    f32 = mybir.dt.float32
    bf16 = mybir.dt.bfloat16
    FT = mybir.ActivationFunctionType

    t_tot = b * s
    KS = h // 128            # contraction subtiles
    TC = 512                 # t-chunk for QKV
    NTC = t_tot // TC
    QT_N = s // 128          # q tiles per batch
    TT_N = t_tot // 128      # t tiles overall
    NCOL = max(1, h // 512)  # dense output column chunks

    nc = bacc.Bacc("TRN2", target_bir_lowering=False, debug=False)

    dynamic = reps == "dynamic"
    nreps = (
        nc.dram_tensor("nreps", [1, 1], mybir.dt.int32, kind="ExternalInput").ap()
        if dynamic
        else None
    )
    hid_t = nc.dram_tensor("hid_t", [h, t_tot], bf16, kind="ExternalInput").ap()
    wqk = nc.dram_tensor("wqk", [h, 2 * NL * HD], bf16, kind="ExternalInput").ap()
    wv = nc.dram_tensor("wv", [h, NL * HD], bf16, kind="ExternalInput").ap()
    wd = nc.dram_tensor("wd", [NL * HD, h], bf16, kind="ExternalInput").ap()
    bqk = nc.dram_tensor("bqk", [HD, 2 * NL], f32, kind="ExternalInput").ap()
    bvd = nc.dram_tensor("bvd", [HD, NL * HD], f32, kind="ExternalInput").ap()
    out = nc.dram_tensor("out_partial", [t_tot, h], bf16, kind="ExternalOutput").ap()

    hid_r = hid_t.rearrange("(ho p) t -> p ho t", p=128)
    wqk_r = wqk.rearrange("(ho p) j -> p ho j", p=128)
    wv_r = wv.rearrange("(ho p) j -> p ho j", p=128)
    wd_r = wd.rearrange("(ho p) n -> p ho n", p=128)
    out_r = out.rearrange("(to p) n -> p to n", p=128)

    with tile.TileContext(nc) as tc:
        with (
            tc.tile_pool(name="const", bufs=1) as const,
            tc.tile_pool(name="hid", bufs=4) as hidp,
            tc.tile_pool(name="work", bufs=3) as work,
            tc.tile_pool(name="psum", bufs=2, space="PSUM") as psum,
        ):
            # ---- constants (loaded once, outside the reps loop) ----
            wqk_sb = const.tile([128, KS, 2 * NL * HD], bf16, tag="wqk")
            wv_sb = const.tile([128, KS, NL * HD], bf16, tag="wv")
            pw = KS // 4
            # only the first wqk piece up-front; the rest go after the first
            # hidden piece so the first matmul can start ASAP
            nc.sync.dma_start(wqk_sb[:, ds(0, pw), :], wqk_r[:, ds(0, pw), :])
            wd_sb = const.tile([128, NL, h], bf16, tag="wd")
            bqk_sb = const.tile([128, 2 * NL], f32, tag="bqk")
            nc.sync.dma_start(bqk_sb[:], bqk)
            bvd_sb = const.tile([128, NL * HD], f32, tag="bvd")
            nc.sync.dma_start(bvd_sb[:], bvd)
            # triangle mask: tri[k, q] = 1 if k <= q else 0 (for diagonal tiles)
            tri_sb = const.tile([128, 128], bf16, tag="tri")
            make_upper_triangular(nc, tri_sb[:], val=1.0, diag=True)
            ident_sb = const.tile([128, 128], bf16, tag="ident")
            make_identity(nc, ident_sb[:])

            QW = 512
            QC_N = s // QW
            NW = min(512, h)

            def body(_i):
                # persistent intermediates (per rep)
                qkT_sb = const.tile([128, 2 * NL, t_tot], bf16, tag="qkT")
                v_sb = const.tile([128, TT_N, NL, HD + 1], bf16, tag="v")
                attnT_sb = const.tile([128, NL, t_tot], bf16, tag="attnT")

                # ones column for fused softmax denominator
                nc.vector.memset(v_sb[:, :, :, HD : HD + 1], 1.0)

                def qkv_chunk(tci):
                    ht = hidp.tile([128, KS, TC], bf16, tag="ht")
                    # split the chunk DMA so the first matmuls can start sooner
                    # (subtile deps let hs-slice consumers fire per piece)
                    nc.sync.dma_start(
                        ht[:, ds(0, KS // 4), :],
                        hid_r[:, ds(0, KS // 4), ts(tci, TC)],
                    )
                    if tci == 0:
                        # remaining weight pieces, after the first hidden piece
                        for pc in range(1, 4):
                            nc.sync.dma_start(
                                wqk_sb[:, ds(pc * pw, pw), :],
                                wqk_r[:, ds(pc * pw, pw), :],
                            )
                    for pc in range(1, 4):
                        nc.sync.dma_start(
                            ht[:, ds(pc * (KS // 4), KS // 4), :],
                            hid_r[:, ds(pc * (KS // 4), KS // 4), ts(tci, TC)],
                        )
                    if tci == 0:
                        for pc in range(4):
                            nc.sync.dma_start(
                                wv_sb[:, ds(pc * pw, pw), :],
                                wv_r[:, ds(pc * pw, pw), :],
                            )
                    # q^T / k^T:  psum[j, t] = sum_h W[h, j] * hidden^T[h, t]
                    for j in range(2 * NL):
                        ps = psum.tile([128, TC], f32, tag="a", bufs=2)
                        for hs in range(KS):
                            nc.tensor.matmul(
                                ps[:],
                                lhsT=wqk_sb[:, hs, ts(j, HD)],
                                rhs=ht[:, hs, :],
                                start=(hs == 0),
                                stop=(hs == KS - 1),
                            )
                        nc.vector.tensor_scalar_add(
                            qkT_sb[:, j, ts(tci, TC)], ps[:], bqk_sb[:, j : j + 1]
                        )
                    # v:  psum[t, j] = sum_h hidden^T[h, t] * Wv[h, j]
                    for tt in range(TC // 128):
                        ps = psum.tile([128, NL * HD], f32, tag="a", bufs=2)
                        for hs in range(KS):
                            nc.tensor.matmul(
                                ps[:],
                                lhsT=ht[:, hs, ts(tt, 128)],
                                rhs=wv_sb[:, hs, :],
                                start=(hs == 0),
                                stop=(hs == KS - 1),
                            )
                        ti = tci * (TC // 128) + tt
                        for hh in range(NL):
                            nc.vector.tensor_tensor(
                                v_sb[:, ti, hh, 0:HD],
                                ps[:, ts(hh, HD)],
                                bvd_sb[:, ts(hh, HD)],
                                mybir.AluOpType.add,
                            )

                def attn_qc(bb, qc):
                    if True:
                        for hh in range(NL):
                            pvs = [
                                psum.tile(
                                    [128, HD + 1], f32, tag="pv", bufs=4,
                                    name=f"pv{j}",
                                )
                                for j in range(4)
                            ]
                            n_kt = 4 * qc + 4
                            for kt in range(n_kt):
                                d_ = kt - 4 * qc
                                q0 = max(d_, 0) * 128  # skip fully-masked q cols
                                sp = psum.tile([128, QW], f32, tag="s", bufs=2)
                                nc.tensor.matmul(
                                    sp[:, q0:QW],
                                    lhsT=qkT_sb[:, NL + hh, ds(bb * s + kt * 128, 128)],
                                    rhs=qkT_sb[:, hh, ds(bb * s + qc * QW + q0, QW - q0)],
                                    start=True,
                                    stop=True,
                                )
                                e = work.tile([128, QW], bf16, tag="e", bufs=8)
                                nc.scalar.activation(
                                    e[:, q0:QW], sp[:, q0:QW], FT.Exp, scale=SCALE
                                )
                                if d_ >= 0:
                                    nc.vector.tensor_tensor(
                                        e[:, ds(q0, 128)],
                                        e[:, ds(q0, 128)],
                                        tri_sb[:],
                                        mybir.AluOpType.mult,
                                    )
                                kg = (bb * s) // 128 + kt
                                for j in range(max(d_, 0), 4):
                                    qt = 4 * qc + j
                                    nc.tensor.matmul(
                                        pvs[j][:],
                                        lhsT=e[:, ts(j, 128)],
                                        rhs=v_sb[:, kg, hh, :],
                                        start=(kt == 0),
                                        stop=(kt == qt),
                                    )
                            for j in range(4):
                                qt = 4 * qc + j
                                rec = work.tile([128, 1], f32, tag="rec", bufs=6)
                                nc.vector.reciprocal(rec[:], pvs[j][:, HD : HD + 1])
                                a_sb = work.tile([128, 128], bf16, tag="attn", bufs=6)
                                nc.vector.tensor_scalar_mul(
                                    a_sb[:], pvs[j][:, 0:HD], rec[:]
                                )
                                tp = psum.tile([128, 128], bf16, tag="s", bufs=2)
                                nc.tensor.transpose(tp[:], a_sb[:], ident_sb[:])
                                nc.vector.tensor_copy(
                                    attnT_sb[:, hh, ds(bb * s + qt * 128, 128)],
                                    tp[:],
                                )

                def dense_tiles(tts, deep=False):
                    # deep=True: attention psum tags are free (tail region) —
                    # rotate across them for a deeper psum pipeline
                    tags = ["a", "pv", "s", "pv"] if deep else ["a"]
                    tagbufs = {"a": 2, "pv": 4, "s": 2}
                    for gi, tt in enumerate(tts):
                        for ncc in range(NCOL):
                            tg = tags[(gi * NCOL + ncc) % len(tags)]
                            dps = psum.tile(
                                [128, NW], f32, tag=tg, bufs=tagbufs[tg], name="dps"
                            )
                            for hh in range(NL):
                                nc.tensor.matmul(
                                    dps[:],
                                    lhsT=attnT_sb[:, hh, ts(tt, 128)],
                                    rhs=wd_sb[:, hh, ts(ncc, NW)],
                                    start=(hh == 0),
                                    stop=(hh == NL - 1),
                                )
                            dst = work.tile([128, NW], bf16, tag="dst", bufs=6)
                            # alternate engines so the psum->sbuf copy rate
                            # doesn't gate the 2-slot psum pipeline
                            if (tt * NCOL + ncc) % 2 == 0:
                                nc.vector.tensor_copy(dst[:], dps[:])
                            else:
                                nc.scalar.copy(dst[:], dps[:])
                            nc.sync.dma_start(out_r[:, tt, ts(ncc, NW)], dst[:])

                # Interleaved emission so ACT-bound attention overlaps PE-bound
                # QKV/dense work (the scheduler's lookahead is bounded, so the
                # interleave must happen at emission order).
                npc = NTC // b  # qkv chunks per batch
                tpb = TT_N // b  # dense t-tiles per batch
                do_qkv = "qkv" in phases
                for tci in range(npc if do_qkv else 0):
                    qkv_chunk(tci)
                    if tci == 0:
                        # deferred: keeps the startup DMA queue clear for
                        # the first hidden/weight pieces
                        nc.sync.dma_start(wd_sb[:], wd_r)
                do_attn = "attn" in phases
                do_dense = "dense" in phases
                for qc in range(QC_N):
                    if do_attn:
                        attn_qc(0, qc)
                    if do_qkv and b > 1 and qc < npc:
                        qkv_chunk(npc + qc)
                if b > 1:
                    for tci in range(npc + QC_N, NTC) if do_qkv else []:
                        qkv_chunk(tci)
                    qtp = tpb // QC_N
                    for qc in range(QC_N):
                        if do_attn:
                            attn_qc(1, qc)
                        if do_dense:
                            dense_tiles(range(qc * qtp, (qc + 1) * qtp))
                            if qc > 0:
                                dense_tiles(
                                    range(tpb + (qc - 1) * qtp, tpb + qc * qtp)
                                )
                    if do_dense:
                        dense_tiles(range(tpb + (QC_N - 1) * qtp, TT_N), deep=do_attn)
                elif do_dense:
                    dense_tiles(range(TT_N))

            if dynamic:
                nrep_sb = const.tile([1, 1], mybir.dt.int32, tag="nreps")
                nc.sync.dma_start(nrep_sb[:], nreps)
                rv = nc.sync.value_load(nrep_sb[:], min_val=1, max_val=100000)
                with tc.For_i(0, rv, 1) as i:
                    body(i)
            elif reps == 1:
                body(0)
            else:
                with tc.For_i(0, reps, 1) as i:
                    body(i)

    nc.compile()
    return nc


def _pack_inputs(hidden_states, W_qkv, b_qkv, W_dense):
    """Per-core input maps (host-side sharding)."""
    hid = np.asarray(hidden_states, dtype=np.float32).reshape(T, H)
    hid_t = np.ascontiguousarray(hid.T).astype(_BF16)
    W_qkv = np.asarray(W_qkv, dtype=np.float32)
    b_qkv = np.asarray(b_qkv, dtype=np.float32)
    W_dense = np.asarray(W_dense, dtype=np.float32)
    Wq, Wk, Wv = W_qkv[:, 0:H], W_qkv[:, H : 2 * H], W_qkv[:, 2 * H : 3 * H]
    bq, bk, bv = b_qkv[0:H], b_qkv[H : 2 * H], b_qkv[2 * H : 3 * H]

    in_maps = []
    for c in range(NCORES):
        hs = [NL * c + i for i in range(NL)]
        cols = [Wq[:, h * HD : (h + 1) * HD] for h in hs] + [
            Wk[:, h * HD : (h + 1) * HD] for h in hs
        ]
        wqk_c = np.concatenate(cols, axis=1).astype(_BF16)
        wv_c = np.concatenate(
            [Wv[:, h * HD : (h + 1) * HD] for h in hs], axis=1
        ).astype(_BF16)
        wd_c = np.ascontiguousarray(
            W_dense[c * NL * HD : (c + 1) * NL * HD, :]
        ).astype(_BF16)
        bqk_c = np.stack(
            [bq[h * HD : (h + 1) * HD] for h in hs]
            + [bk[h * HD : (h + 1) * HD] for h in hs],
            axis=1,
        ).astype(np.float32)
        bv_row = np.concatenate([bv[h * HD : (h + 1) * HD] for h in hs])
        bvd_c = np.ascontiguousarray(
            np.broadcast_to(bv_row[None, :], (HD, NL * HD))
        ).astype(np.float32)
        in_maps.append(
            {
                "hid_t": hid_t,
                "wqk": wqk_c,
                "wv": wv_c,
                "wd": wd_c,
                "bqk": bqk_c,
                "bvd": bvd_c,
            }
        )
    return in_maps


def make_runner(nc, n_cores=NCORES):
    """Reusable jitted SPMD runner (no donation; device-resident inputs)."""
    import jax
    import concourse.mybir as mybir
    from jax.sharding import Mesh, PartitionSpec
    from jax.experimental.shard_map import shard_map
    from concourse.bass2jax import (
        _bass_exec_p,
        partition_id_tensor,
        install_neuronx_cc_hook,
    )

    install_neuronx_cc_hook()
    partition_name = nc.partition_id_tensor.name if nc.partition_id_tensor else None
    in_names, out_names, out_avals = [], [], []
    for alloc in nc.m.functions[0].allocations:
        if not isinstance(alloc, mybir.MemoryLocationSet):
            continue
        name = alloc.memorylocations[0].name
        if alloc.kind == "ExternalInput":
            if name != partition_name:
                in_names.append(name)
        elif alloc.kind == "ExternalOutput":
            out_names.append(name)
            out_avals.append(
                jax.core.ShapedArray(
                    tuple(alloc.tensor_shape), mybir.dt.np(alloc.dtype)
                )
            )
    n_params = len(in_names)
    all_in_names = list(in_names) + list(out_names)
    if partition_name is not None:
        all_in_names.append(partition_name)
    zero_outs = [np.zeros(a.shape, a.dtype) for a in out_avals]

    def _body(*args):
        operands = list(args)
        if partition_name is not None:
            operands.append(partition_id_tensor())
        outs = _bass_exec_p.bind(
            *operands,
            out_avals=tuple(out_avals),
            in_names=tuple(all_in_names),
            out_names=tuple(out_names),
            lowering_input_output_aliases=(),
            sim_require_finite=True,
            sim_require_nnan=True,
            nc=nc,
        )
        return tuple(outs)

    devices = jax.devices()[:n_cores]
    mesh = Mesh(np.asarray(devices), ("core",))
    in_specs = (PartitionSpec("core"),) * (n_params + len(out_names))
    out_specs = (PartitionSpec("core"),) * len(out_names)
    sharded = jax.jit(
        shard_map(
            _body, mesh=mesh, in_specs=in_specs, out_specs=out_specs, check_rep=False
        ),
        keep_unused=True,
    )

    def prepare(in_maps):
        per_core = [[np.asarray(m[name]) for name in in_names] for m in in_maps]
        concat_in = [
            np.concatenate([per_core[c][i] for c in range(n_cores)], axis=0)
            for i in range(n_params)
        ]
        concat_zero = [
            np.zeros((n_cores * z.shape[0], *z.shape[1:]), z.dtype) for z in zero_outs
        ]
        return [jax.device_put(a) for a in concat_in + concat_zero]

    def run(dev_args):
        outs = sharded(*dev_args)
        jax.block_until_ready(outs)
        return outs

    def fetch(outs):
        return [
            {
                name: np.asarray(outs[i]).reshape(n_cores, *out_avals[i].shape)[c]
                for i, name in enumerate(out_names)
            }
            for c in range(n_cores)
        ]

    return prepare, run, fetch


def kernel(hidden_states, W_qkv, b_qkv, W_dense, b_dense):
    from concourse import bass_utils

    if "nc1" not in _CACHE:
        _CACHE["nc1"] = _build_nc(reps=1)
    nc = _CACHE["nc1"]

    in_maps = _pack_inputs(hidden_states, W_qkv, b_qkv, W_dense)
    res = bass_utils.run_bass_kernel_spmd(nc, in_maps, core_ids=list(range(NCORES)))

    acc = np.zeros((T, H), dtype=np.float64)
    for c in range(NCORES):
        acc += res.results[c]["out_partial"].astype(np.float64)
    acc += np.asarray(b_dense, dtype=np.float64)[None, :]
    return acc.astype(np.float32).reshape(B, S, H)
